# revision 67
# baseline (speedup 1.0000x reference)
"""Trainium2 Bass kernel for BinarySphericalQuantizer (vq_codebook).

Full inputs in, full outputs out. Internally: pure data-parallel over 8
NeuronCores (2 batch images per core); tiny cross-shard reductions
(avg_prob, entropy/commit scalars) are finished on the host.

Per-core device algorithm (all f32, token-major):
  z shard viewed as [36, 4096]  (partition = (image, channel), free = h*w)
  zhat   = Sign(z)                                  -> zq output
  sqpart = rowsum (|z|-1)^2                         -> commit loss partials
  idxraw = basisT @ zhat  (PE, stationary basis)    -> indices (host affine)
  per 128-token tile j, group g:
    logits  = (2*cb_blockdiag)T-slice . z-tile      (PE -> PSUM [128,1024])
    e       = Exp(logits), Z = rowsum e             (ACT, fused accum_out)
    Sel     = rowsum e*logits                       (DVE tensor_tensor_reduce)
    r       = 1/Z                                   (DVE reciprocal)
    probsum += r-col.T @ e                          (PE, PSUM accumulate)
  lpart = rowsum ln(Zbuf), npart = rowsum Sel*r     (ACT/DVE, fused accums)
Host: entropy = sum(lpart) - sum(npart); avg_prob = sum(probsum)/Ntok; etc.
"""

import os
import sys
from contextlib import ExitStack

import numpy as np

sys.path.insert(0, "/opt/trn_rl_repo")

import concourse.bass as bass
import concourse.bacc as bacc
import concourse.mybir as mybir
import concourse.tile as tile
from concourse.bass_utils import run_bass_kernel_spmd

F32 = mybir.dt.float32
BF16 = mybir.dt.bfloat16
AF = mybir.ActivationFunctionType
OP = mybir.AluOpType

B, C, HW = 16, 18, 64 * 64  # full problem
NCORES = 8
BPC = B // NCORES            # batch images per core (2)
P = 36                       # BPC * C partitions
TOK = BPC * HW               # tokens per core (8192)
NTILE = TOK // 128           # 64 token tiles per core
G = 9
D = 512                      # codes per group
EPS_LOG = 1e-8
BETA = 0.25


def _consts():
    codes = np.arange(D)[:, None]
    bits = (codes // (2 ** np.arange(G - 1, -1, -1))[None, :]) % 2
    gcb = (bits * 2 - 1).astype(np.float32)          # [512, 9]
    cbrhs = np.zeros((C, 2 * D), np.float32)         # pre-scaled by 2 (logits = 2 z.cb)
    cbrhs[0:G, 0:D] = 2.0 * gcb.T
    cbrhs[G:C, D : 2 * D] = 2.0 * gcb.T
    basis3 = np.zeros((C, 3), np.float32)
    basis3[:, 0] = 2.0 ** np.arange(C - 1, -1, -1)
    basis3[0:G, 1] = 2.0 ** np.arange(G - 1, -1, -1)
    basis3[G:C, 2] = 2.0 ** np.arange(G - 1, -1, -1)
    return cbrhs, basis3


FREE_IN = HW + 2 * D + 3  # [z | cbrhs | basis3] per image row-block


def build_nc(stage=99):
    nc = bacc.Bacc("TRN2", target_bir_lowering=False)
    zin_d = nc.dram_tensor("zin", [64, FREE_IN], BF16, kind="ExternalInput")
    zq_d = nc.dram_tensor("zq", [P, HW], F32, kind="ExternalOutput")
    idx_d = nc.dram_tensor("idxraw", [128, 3 * NTILE], F32, kind="ExternalOutput")
    ps_d = nc.dram_tensor("probsum", [2, 2 * D], F32, kind="ExternalOutput")
    lp_d = nc.dram_tensor("lpart", [128, 1], F32, kind="ExternalOutput")
    np_d = nc.dram_tensor("npart", [128, 1], F32, kind="ExternalOutput")
    sq_d = nc.dram_tensor("sqpart", [64, 2], F32, kind="ExternalOutput")

    with ExitStack() as ctx:
        tc = ctx.enter_context(tile.TileContext(nc))
        persist = ctx.enter_context(tc.tile_pool(name="persist", bufs=1))
        epool = ctx.enter_context(tc.tile_pool(name="epool", bufs=4))
        elpool = ctx.enter_context(tc.tile_pool(name="elpool", bufs=3))
        lpsum = ctx.enter_context(tc.tile_pool(name="lpsum", bufs=5, space="PSUM"))
        ipsum = ctx.enter_context(tc.tile_pool(name="ipsum", bufs=1, space="PSUM"))
        ppsum = ctx.enter_context(tc.tile_pool(name="ppsum", bufs=1, space="PSUM"))

        # images live at partition offsets 0 and 32 (matmul base-partition
        # constraint: lhsT/rhs/out bases must be 0/32/64); rows 18..31 and
        # 50..63 hold garbage that no matmul reads (host masks sqpart rows).
        # Each logical load is ONE dma_start so downstream matmuls carry at
        # most 2 sync waits (LDWEIGHTS has very few wait slots).
        # ONE input DMA for everything (z, codebook, basis, zeroed filler
        # rows assembled host-side) -> downstream instructions carry at most
        # one DMA wait (LDWEIGHTS has very few sync-wait slots)
        zin = persist.tile([64, FREE_IN], BF16)
        nc.sync.dma_start(out=zin[:, :], in_=zin_d[:, :])
        z36 = zin[:, 0:HW]
        cb = zin[:, HW : HW + 2 * D]
        b3 = zin[:, HW + 2 * D : HW + 2 * D + 3]

        if stage < 1:
            nc.compile()
            return nc
        # zq = sign(z)  (garbage partition rows 18..31/50..63 are never read
        # by matmuls; sqpart garbage rows are masked on the host).
        # ACT produces only the bf16 copy (index-matmul weights; +-1 exact);
        # the f32 copy for the zq output is upconverted on idle GPSIMD.
        # sign via bf16 bit ops on idle GPSIMD: (z & 0x8000) | 0x3f80 = +-1.0
        # (keeps ACT free for the exp stream; z==+0 -> +1 is unobservable for
        # continuous gaussian input)
        zhatb = persist.tile([64, HW], BF16)
        nc.vector.tensor_scalar(
            out=zhatb[:, :].bitcast(mybir.dt.uint16),
            in0=z36.bitcast(mybir.dt.uint16),
            scalar1=0x8000, scalar2=0x3F80,
            op0=OP.bitwise_and, op1=OP.bitwise_or,
        )
        zhat = persist.tile([64, HW], F32)
        nc.gpsimd.tensor_copy(zhat[:, :], zhatb[:, :])
        for im in range(BPC):
            nc.sync.dma_start(
                out=zq_d[C * im : C * im + C, :],
                in_=zhat[32 * im : 32 * im + C, :],
            )

        if stage < 2:
            nc.compile()
            return nc
        # commit-loss partials on DVE (stt is not a legal Pool opcode):
        # rowsum (|z|-1)^2 = rowsum z^2 - 2*rowsum|z| + HW  (host combines);
        # |z| = z * sign(z), both exact for bf16 z in the f32 accumulator
        sqp = persist.tile([64, 2], F32)
        sqtr = persist.tile([64, HW], BF16)
        nc.vector.scalar_tensor_tensor(
            out=sqtr[:, :], in0=z36, scalar=1.0, in1=z36,
            op0=OP.mult, op1=OP.mult, accum_out=sqp[:, 0:1],
        )
        nc.vector.scalar_tensor_tensor(
            out=sqtr[:, :], in0=z36, scalar=1.0, in1=zhatb[:, :],
            op0=OP.mult, op1=OP.mult, accum_out=sqp[:, 1:2],
        )
        nc.sync.dma_start(out=sq_d[:, :], in_=sqp[:, :])

        if stage < 3:
            nc.compile()
            return nc
        # main softmax/entropy/probsum loop
        Zbuf = persist.tile([128, 2 * NTILE], F32)
        Sel = persist.tile([128, 2 * NTILE], F32)
        rbuf = persist.tile([128, 2 * NTILE], BF16)
        # er-matmuls use both r columns as weights; row g of pp[g] is the
        # wanted probsum row, the other row is a discarded cross term
        pp = [
            ppsum.tile([2, D], F32, tag=f"pp{g}", name=f"pp{g}") for g in range(2)
        ]
        # token-major raw index dot products, packed [128, 3*NTILE]
        pidx = ipsum.tile([128, 3 * NTILE], F32)

        for j in range(NTILE):
            im, jj = divmod(j, NTILE // BPC)
            ztile = z36[32 * im : 32 * im + C, jj * 128 : (jj + 1) * 128]
            cbi = cb[32 * im : 32 * im + C, :]
            # one single-bank PSUM tile per group, 5-deep pipelined
            plg = [
                lpsum.tile([128, D], F32, tag="pl", name=f"pl{j}_{g}")
                for g in range(2)
            ]
            for g in range(2):
                nc.tensor.matmul(
                    out=plg[g][:, :],
                    lhsT=ztile,
                    rhs=cbi[:, g * D : (g + 1) * D],
                    start=True,
                    stop=True,
                )
            nc.tensor.matmul(
                out=pidx[:, 3 * j : 3 * j + 3],
                lhsT=zhatb[32 * im : 32 * im + C, jj * 128 : (jj + 1) * 128],
                rhs=b3[32 * im : 32 * im + C, :],
                start=True,
                stop=True,
                skip_group_check=True,
            )
            if stage < 4:
                continue
            e = epool.tile([128, 2 * D], BF16)
            for g in range(2):
                col = 2 * j + g
                nc.scalar.activation(
                    out=e[:, g * D : (g + 1) * D],
                    in_=plg[g][:, :],
                    func=AF.Exp,
                    accum_out=Zbuf[:, col : col + 1],
                )
            if stage < 5:
                continue
            with nc.allow_low_precision("bf16 1/Z weights for probsum matmul"):
                nc.vector.reciprocal(
                    out=rbuf[:, 2 * j : 2 * j + 2], in_=Zbuf[:, 2 * j : 2 * j + 2]
                )
            for g in range(2):
                col = 2 * j + g
                el = elpool.tile([128, D], BF16)
                # el = e * logits, Sel col = rowsum(el)  (tensor_tensor_reduce
                # hard-crashes the device on this ucode; scalar_tensor_tensor
                # with a bypassed scalar stage does the same fused job)
                nc.vector.scalar_tensor_tensor(
                    out=el[:, :],
                    in0=plg[g][:, :],
                    scalar=1.0,
                    in1=e[:, g * D : (g + 1) * D],
                    op0=OP.mult,
                    op1=OP.mult,
                    accum_out=Sel[:, col : col + 1],
                )
                if stage < 6:
                    continue
                nc.tensor.matmul(
                    out=pp[g][:, :],
                    lhsT=rbuf[:, 2 * j : 2 * j + 2],
                    rhs=e[:, g * D : (g + 1) * D],
                    start=(j == 0),
                    stop=(j == NTILE - 1),
                    skip_group_check=True,
                )

        if stage < 7:
            nc.compile()
            return nc
        # entropy partials: sum_cols ln Z  and  sum_cols Sel/Z
        Lbuf = persist.tile([128, 2 * NTILE], F32)
        lp = persist.tile([128, 1], F32)
        nc.scalar.activation(
            out=Lbuf[:, :], in_=Zbuf[:, :], func=AF.Ln, accum_out=lp[:, :]
        )
        # f32 1/Z for the entropy tail (bf16 rbuf noise would be amplified
        # by the lnZ - Sel/Z cancellation)
        rf32 = persist.tile([128, 2 * NTILE], F32)
        nc.vector.reciprocal(out=rf32[:, :], in_=Zbuf[:, :])
        T1 = persist.tile([128, 2 * NTILE], F32)
        npart = persist.tile([128, 1], F32)
        nc.vector.scalar_tensor_tensor(
            out=T1[:, :],
            in0=Sel[:, :],
            scalar=1.0,
            in1=rf32[:, :],
            op0=OP.mult,
            op1=OP.mult,
            accum_out=npart[:, :],
        )
        nc.sync.dma_start(out=lp_d[:, :], in_=lp[:, :])
        nc.sync.dma_start(out=np_d[:, :], in_=npart[:, :])
        ppsb = persist.tile([2, 2 * D], F32)
        for g in range(2):
            nc.vector.tensor_copy(ppsb[0:2, g * D : (g + 1) * D], pp[g][0:2, :])
        nc.sync.dma_start(out=ps_d[:, :], in_=ppsb[:, :])
        idxsb = persist.tile([128, 3 * NTILE], F32)
        nc.vector.tensor_copy(idxsb[:, :], pidx[:, :])
        nc.sync.dma_start(out=idx_d[:, :], in_=idxsb[:, :])

    nc.compile()
    return nc


_NC = None


def _get_nc():
    global _NC
    if _NC is None:
        _NC = build_nc()
    return _NC


def make_zin(z):
    """Assemble per-core [64, FREE_IN] bf16 input blocks: [z | cbrhs | basis3]."""
    import ml_dtypes

    cbrhs, basis3 = _consts()
    zf = np.ascontiguousarray(z, dtype=np.float32).reshape(NCORES, BPC, C, HW)
    zin = np.zeros((NCORES, 64, FREE_IN), ml_dtypes.bfloat16)
    for im in range(BPC):
        rows = slice(32 * im, 32 * im + C)
        zin[:, rows, 0:HW] = zf[:, im].astype(ml_dtypes.bfloat16)
        zin[:, rows, HW : HW + 2 * D] = cbrhs.astype(ml_dtypes.bfloat16)
        zin[:, rows, HW + 2 * D :] = basis3.astype(ml_dtypes.bfloat16)
    return zin


def run_device(z, trace=False, **kw):
    """z: full [16,18,64,64] f32. Returns (per-core results list, BassKernelResults)."""
    zin = make_zin(z)
    in_maps = [{"zin": zin[i]} for i in range(NCORES)]
    nc = _get_nc()
    br = run_bass_kernel_spmd(nc, in_maps, list(range(NCORES)), trace=trace, **kw)
    return br.results, br


def finish_host(results):
    """Combine per-core outputs into the reference's 6-tuple."""
    zq = np.stack([r["zq"] for r in results]).reshape(B, C, 64, 64)
    # [8, 128, 3*NTILE] -> [8, 3, TOK]: tile j cols 3j..3j+3, token = j*128 + p
    idxraw = (
        np.stack([r["idxraw"] for r in results])
        .reshape(NCORES, 128, NTILE, 3)
        .transpose(0, 3, 2, 1)
        .reshape(NCORES, 3, TOK)
    )
    # [2, 1024]: row g of column-block g is the real probsum row for group g
    probsum = np.stack(
        [
            np.sum([r["probsum"][g, g * D : (g + 1) * D] for r in results], axis=0, dtype=np.float64)
            for g in range(2)
        ]
    )
    lsum = np.sum([r["lpart"] for r in results], dtype=np.float64)
    nsum = np.sum([r["npart"] for r in results], dtype=np.float64)
    # sqpart rows: [rowsum z^2, rowsum |z|]; (|z|-1)^2 = z^2 - 2|z| + 1
    sqvalid = np.r_[0:C, 32 : 32 + C]
    sq2 = np.stack([r["sqpart"][sqvalid] for r in results]).astype(np.float64)
    sqsum = sq2[:, :, 0].sum() - 2.0 * sq2[:, :, 1].sum() + NCORES * 2 * C * HW

    ntok = B * HW
    avg_prob = (probsum / ntok).astype(np.float32)           # [2, 512]
    per_sample_entropy = np.float32((lsum - nsum) / ntok)
    cb_entropy = np.float32(-(avg_prob * np.log(avg_prob + EPS_LOG)).sum())
    commit_loss = np.float32(BETA * sqsum / ntok)
    loss = np.float32(commit_loss + per_sample_entropy - cb_entropy)

    flat = (0.5 * idxraw[:, 0] + (2.0 ** C - 1) / 2.0)
    g0 = 0.5 * idxraw[:, 1] + (2.0 ** G - 1) / 2.0
    g1 = 0.5 * idxraw[:, 2] + (2.0 ** G - 1) / 2.0
    indices = np.rint(flat).astype(np.int32).reshape(B, 64, 64)
    group_indices = (
        np.rint(np.stack([g0, g1], axis=-1)).astype(np.int32).reshape(B, 64, 64, 2)
    )
    return zq, loss, cb_entropy, indices, group_indices, avg_prob


def kernel(z):
    results, _ = run_device(z, trace=False)
    return finish_host(results)


# revision 70
# speedup vs baseline: 1.0196x; 1.0196x over previous
"""Trainium2 Bass kernel for BinarySphericalQuantizer (vq_codebook).

Full inputs in, full outputs out. Internally: pure data-parallel over 8
NeuronCores (2 batch images per core); tiny cross-shard reductions
(avg_prob, entropy/commit scalars) are finished on the host.

Per-core device algorithm (all f32, token-major):
  z shard viewed as [36, 4096]  (partition = (image, channel), free = h*w)
  zhat   = Sign(z)                                  -> zq output
  sqpart = rowsum (|z|-1)^2                         -> commit loss partials
  idxraw = basisT @ zhat  (PE, stationary basis)    -> indices (host affine)
  per 128-token tile j, group g:
    logits  = (2*cb_blockdiag)T-slice . z-tile      (PE -> PSUM [128,1024])
    e       = Exp(logits), Z = rowsum e             (ACT, fused accum_out)
    Sel     = rowsum e*logits                       (DVE tensor_tensor_reduce)
    r       = 1/Z                                   (DVE reciprocal)
    probsum += r-col.T @ e                          (PE, PSUM accumulate)
  lpart = rowsum ln(Zbuf), npart = rowsum Sel*r     (ACT/DVE, fused accums)
Host: entropy = sum(lpart) - sum(npart); avg_prob = sum(probsum)/Ntok; etc.
"""

import os
import sys
from contextlib import ExitStack

import numpy as np

sys.path.insert(0, "/opt/trn_rl_repo")

import concourse.bass as bass
import concourse.bacc as bacc
import concourse.mybir as mybir
import concourse.tile as tile
from concourse.bass_utils import run_bass_kernel_spmd

F32 = mybir.dt.float32
BF16 = mybir.dt.bfloat16
AF = mybir.ActivationFunctionType
OP = mybir.AluOpType

B, C, HW = 16, 18, 64 * 64  # full problem
NCORES = 8
BPC = B // NCORES            # batch images per core (2)
P = 36                       # BPC * C partitions
TOK = BPC * HW               # tokens per core (8192)
NTILE = TOK // 128           # 64 token tiles per core
G = 9
D = 512                      # codes per group
EPS_LOG = 1e-8
BETA = 0.25


def _consts():
    codes = np.arange(D)[:, None]
    bits = (codes // (2 ** np.arange(G - 1, -1, -1))[None, :]) % 2
    gcb = (bits * 2 - 1).astype(np.float32)          # [512, 9]
    cbrhs = np.zeros((C, 2 * D), np.float32)         # pre-scaled by 2 (logits = 2 z.cb)
    cbrhs[0:G, 0:D] = 2.0 * gcb.T
    cbrhs[G:C, D : 2 * D] = 2.0 * gcb.T
    basis3 = np.zeros((C, 3), np.float32)
    basis3[:, 0] = 2.0 ** np.arange(C - 1, -1, -1)
    basis3[0:G, 1] = 2.0 ** np.arange(G - 1, -1, -1)
    basis3[G:C, 2] = 2.0 ** np.arange(G - 1, -1, -1)
    return cbrhs, basis3


FREE_IN = HW + 2 * D + 3  # [z | cbrhs | basis3] per image row-block


def build_nc(stage=99):
    nc = bacc.Bacc("TRN2", target_bir_lowering=False)
    zin_d = nc.dram_tensor("zin", [64, FREE_IN], BF16, kind="ExternalInput")
    zq_d = nc.dram_tensor("zq", [P, HW], F32, kind="ExternalOutput")
    idx_d = nc.dram_tensor("idxraw", [128, 3 * NTILE], F32, kind="ExternalOutput")
    ps_d = nc.dram_tensor("probsum", [2, 2 * D], F32, kind="ExternalOutput")
    lp_d = nc.dram_tensor("lpart", [128, 1], F32, kind="ExternalOutput")
    np_d = nc.dram_tensor("npart", [128, 1], F32, kind="ExternalOutput")
    sq_d = nc.dram_tensor("sqpart", [64, 2], F32, kind="ExternalOutput")

    with ExitStack() as ctx:
        tc = ctx.enter_context(tile.TileContext(nc))
        persist = ctx.enter_context(tc.tile_pool(name="persist", bufs=1))
        epool = ctx.enter_context(tc.tile_pool(name="epool", bufs=4))
        elpool = ctx.enter_context(tc.tile_pool(name="elpool", bufs=3))
        lpsum = ctx.enter_context(tc.tile_pool(name="lpsum", bufs=5, space="PSUM"))
        ipsum = ctx.enter_context(tc.tile_pool(name="ipsum", bufs=1, space="PSUM"))
        ppsum = ctx.enter_context(tc.tile_pool(name="ppsum", bufs=1, space="PSUM"))

        # images live at partition offsets 0 and 32 (matmul base-partition
        # constraint: lhsT/rhs/out bases must be 0/32/64); rows 18..31 and
        # 50..63 hold garbage that no matmul reads (host masks sqpart rows).
        # Each logical load is ONE dma_start so downstream matmuls carry at
        # most 2 sync waits (LDWEIGHTS has very few wait slots).
        # ONE input DMA for everything (z, codebook, basis, zeroed filler
        # rows assembled host-side) -> downstream instructions carry at most
        # one DMA wait (LDWEIGHTS has very few sync-wait slots)
        zin = persist.tile([64, FREE_IN], BF16)
        nc.sync.dma_start(out=zin[:, :], in_=zin_d[:, :])
        z36 = zin[:, 0:HW]
        cb = zin[:, HW : HW + 2 * D]
        b3 = zin[:, HW + 2 * D : HW + 2 * D + 3]

        # sign/zq/commit-loss side work is emitted MID-LOOP (at j==40) so
        # DVE feeds the el-chain immediately at startup; see emit_side() below.
        zhatb = persist.tile([64, HW], BF16)
        zhat = persist.tile([64, HW], F32)
        sqp = persist.tile([64, 2], F32)
        sqtr = persist.tile([64, HW], BF16)

        def emit_side():
            # sign via bf16 bit ops in ONE 4x-mode DVE op:
            # (z & 0x8000) | 0x3f80 = +-1.0  (z==+0 -> +1 unobservable)
            nc.vector.tensor_scalar(
                out=zhatb[:, :].bitcast(mybir.dt.uint16),
                in0=z36.bitcast(mybir.dt.uint16),
                scalar1=0x8000, scalar2=0x3F80,
                op0=OP.bitwise_and, op1=OP.bitwise_or,
            )
            nc.gpsimd.tensor_copy(zhat[:, :], zhatb[:, :])
            for im in range(BPC):
                nc.sync.dma_start(
                    out=zq_d[C * im : C * im + C, :],
                    in_=zhat[32 * im : 32 * im + C, :],
                )
            # commit loss: rowsum (|z|-1)^2 = rowsum z^2 - 2*rowsum|z| + HW
            nc.vector.scalar_tensor_tensor(
                out=sqtr[:, :], in0=z36, scalar=1.0, in1=z36,
                op0=OP.mult, op1=OP.mult, accum_out=sqp[:, 0:1],
            )
            nc.vector.scalar_tensor_tensor(
                out=sqtr[:, :], in0=z36, scalar=1.0, in1=zhatb[:, :],
                op0=OP.mult, op1=OP.mult, accum_out=sqp[:, 1:2],
            )
            nc.sync.dma_start(out=sq_d[:, :], in_=sqp[:, :])
        # main softmax/entropy/probsum loop
        Zbuf = persist.tile([128, 2 * NTILE], F32)
        Sel = persist.tile([128, 2 * NTILE], F32)
        rbuf = persist.tile([128, 2 * NTILE], BF16)
        # er-matmuls use both r columns as weights; row g of pp[g] is the
        # wanted probsum row, the other row is a discarded cross term
        pp = [
            ppsum.tile([2, D], F32, tag=f"pp{g}", name=f"pp{g}") for g in range(2)
        ]
        # token-major raw index dot products, packed [128, 3*NTILE]
        pidx = ipsum.tile([128, 3 * NTILE], F32)

        for j in range(NTILE):
            im, jj = divmod(j, NTILE // BPC)
            ztile = z36[32 * im : 32 * im + C, jj * 128 : (jj + 1) * 128]
            cbi = cb[32 * im : 32 * im + C, :]
            # one single-bank PSUM tile per group, 5-deep pipelined
            plg = [
                lpsum.tile([128, D], F32, tag="pl", name=f"pl{j}_{g}")
                for g in range(2)
            ]
            for g in range(2):
                nc.tensor.matmul(
                    out=plg[g][:, :],
                    lhsT=ztile,
                    rhs=cbi[:, g * D : (g + 1) * D],
                    start=True,
                    stop=True,
                )
            if stage < 4:
                continue
            if j == 40:
                emit_side()
            e = epool.tile([128, 2 * D], BF16)
            for g in range(2):
                col = 2 * j + g
                nc.scalar.activation(
                    out=e[:, g * D : (g + 1) * D],
                    in_=plg[g][:, :],
                    func=AF.Exp,
                    accum_out=Zbuf[:, col : col + 1],
                )
            if stage < 5:
                continue
            with nc.allow_low_precision("bf16 1/Z weights for probsum matmul"):
                nc.vector.reciprocal(
                    out=rbuf[:, 2 * j : 2 * j + 2], in_=Zbuf[:, 2 * j : 2 * j + 2]
                )
            for g in range(2):
                col = 2 * j + g
                el = elpool.tile([128, D], BF16)
                # el = e * logits, Sel col = rowsum(el)  (tensor_tensor_reduce
                # hard-crashes the device on this ucode; scalar_tensor_tensor
                # with a bypassed scalar stage does the same fused job)
                nc.vector.scalar_tensor_tensor(
                    out=el[:, :],
                    in0=plg[g][:, :],
                    scalar=1.0,
                    in1=e[:, g * D : (g + 1) * D],
                    op0=OP.mult,
                    op1=OP.mult,
                    accum_out=Sel[:, col : col + 1],
                )
                if stage < 6:
                    continue
                nc.tensor.matmul(
                    out=pp[g][:, :],
                    lhsT=rbuf[:, 2 * j : 2 * j + 2],
                    rhs=e[:, g * D : (g + 1) * D],
                    start=(j == 0),
                    stop=(j == NTILE - 1),
                    skip_group_check=True,
                )

        if stage < 7:
            nc.compile()
            return nc
        # index matmuls at the PE tail (depend on mid-loop sign)
        for j in range(NTILE):
            im, jj = divmod(j, NTILE // BPC)
            nc.tensor.matmul(
                out=pidx[:, 3 * j : 3 * j + 3],
                lhsT=zhatb[32 * im : 32 * im + C, jj * 128 : (jj + 1) * 128],
                rhs=b3[32 * im : 32 * im + C, :],
                start=True,
                stop=True,
                skip_group_check=True,
            )

        # entropy partials: sum_cols ln Z  and  sum_cols Sel/Z
        Lbuf = persist.tile([128, 2 * NTILE], F32)
        lp = persist.tile([128, 1], F32)
        nc.scalar.activation(
            out=Lbuf[:, :], in_=Zbuf[:, :], func=AF.Ln, accum_out=lp[:, :]
        )
        # f32 1/Z for the entropy tail (bf16 rbuf noise would be amplified
        # by the lnZ - Sel/Z cancellation)
        rf32 = persist.tile([128, 2 * NTILE], F32)
        nc.vector.reciprocal(out=rf32[:, :], in_=Zbuf[:, :])
        T1 = persist.tile([128, 2 * NTILE], F32)
        npart = persist.tile([128, 1], F32)
        nc.vector.scalar_tensor_tensor(
            out=T1[:, :],
            in0=Sel[:, :],
            scalar=1.0,
            in1=rf32[:, :],
            op0=OP.mult,
            op1=OP.mult,
            accum_out=npart[:, :],
        )
        nc.sync.dma_start(out=lp_d[:, :], in_=lp[:, :])
        nc.sync.dma_start(out=np_d[:, :], in_=npart[:, :])
        ppsb = persist.tile([2, 2 * D], F32)
        for g in range(2):
            nc.vector.tensor_copy(ppsb[0:2, g * D : (g + 1) * D], pp[g][0:2, :])
        nc.sync.dma_start(out=ps_d[:, :], in_=ppsb[:, :])
        idxsb = persist.tile([128, 3 * NTILE], F32)
        nc.vector.tensor_copy(idxsb[:, :], pidx[:, :])
        nc.sync.dma_start(out=idx_d[:, :], in_=idxsb[:, :])

    nc.compile()
    return nc


_NC = None


def _get_nc():
    global _NC
    if _NC is None:
        _NC = build_nc()
    return _NC


def make_zin(z):
    """Assemble per-core [64, FREE_IN] bf16 input blocks: [z | cbrhs | basis3]."""
    import ml_dtypes

    cbrhs, basis3 = _consts()
    zf = np.ascontiguousarray(z, dtype=np.float32).reshape(NCORES, BPC, C, HW)
    zin = np.zeros((NCORES, 64, FREE_IN), ml_dtypes.bfloat16)
    for im in range(BPC):
        rows = slice(32 * im, 32 * im + C)
        zin[:, rows, 0:HW] = zf[:, im].astype(ml_dtypes.bfloat16)
        zin[:, rows, HW : HW + 2 * D] = cbrhs.astype(ml_dtypes.bfloat16)
        zin[:, rows, HW + 2 * D :] = basis3.astype(ml_dtypes.bfloat16)
    return zin


def run_device(z, trace=False, **kw):
    """z: full [16,18,64,64] f32. Returns (per-core results list, BassKernelResults)."""
    zin = make_zin(z)
    in_maps = [{"zin": zin[i]} for i in range(NCORES)]
    nc = _get_nc()
    br = run_bass_kernel_spmd(nc, in_maps, list(range(NCORES)), trace=trace, **kw)
    return br.results, br


def finish_host(results):
    """Combine per-core outputs into the reference's 6-tuple."""
    zq = np.stack([r["zq"] for r in results]).reshape(B, C, 64, 64)
    # [8, 128, 3*NTILE] -> [8, 3, TOK]: tile j cols 3j..3j+3, token = j*128 + p
    idxraw = (
        np.stack([r["idxraw"] for r in results])
        .reshape(NCORES, 128, NTILE, 3)
        .transpose(0, 3, 2, 1)
        .reshape(NCORES, 3, TOK)
    )
    # [2, 1024]: row g of column-block g is the real probsum row for group g
    probsum = np.stack(
        [
            np.sum([r["probsum"][g, g * D : (g + 1) * D] for r in results], axis=0, dtype=np.float64)
            for g in range(2)
        ]
    )
    lsum = np.sum([r["lpart"] for r in results], dtype=np.float64)
    nsum = np.sum([r["npart"] for r in results], dtype=np.float64)
    # sqpart rows: [rowsum z^2, rowsum |z|]; (|z|-1)^2 = z^2 - 2|z| + 1
    sqvalid = np.r_[0:C, 32 : 32 + C]
    sq2 = np.stack([r["sqpart"][sqvalid] for r in results]).astype(np.float64)
    sqsum = sq2[:, :, 0].sum() - 2.0 * sq2[:, :, 1].sum() + NCORES * 2 * C * HW

    ntok = B * HW
    avg_prob = (probsum / ntok).astype(np.float32)           # [2, 512]
    per_sample_entropy = np.float32((lsum - nsum) / ntok)
    cb_entropy = np.float32(-(avg_prob * np.log(avg_prob + EPS_LOG)).sum())
    commit_loss = np.float32(BETA * sqsum / ntok)
    loss = np.float32(commit_loss + per_sample_entropy - cb_entropy)

    flat = (0.5 * idxraw[:, 0] + (2.0 ** C - 1) / 2.0)
    g0 = 0.5 * idxraw[:, 1] + (2.0 ** G - 1) / 2.0
    g1 = 0.5 * idxraw[:, 2] + (2.0 ** G - 1) / 2.0
    indices = np.rint(flat).astype(np.int32).reshape(B, 64, 64)
    group_indices = (
        np.rint(np.stack([g0, g1], axis=-1)).astype(np.int32).reshape(B, 64, 64, 2)
    )
    return zq, loss, cb_entropy, indices, group_indices, avg_prob


def kernel(z):
    results, _ = run_device(z, trace=False)
    return finish_host(results)
